# revision 41
# baseline (speedup 1.0000x reference)
"""Trainium2 Bass kernel for nn_MetaLearner (meta-learning attention + cosine
prototype scoring), data-parallel over tasks on 8 NeuronCores.

Math (per task):
  c   = [img, txt] @ Wc.T + bc                (Wc = concat(Wi, Wt))
  h   = LN1(c);  q,k,v = h @ W{q,k,v}.T + b   (queries: seqlen=1 -> ctx = v)
  ctx = softmax(q k^T / sqrt(128)) v          (support: seqlen=4)
  f   = LN2(ctx) @ Wo.T + bo
  logits[t,q,c] = 10 * cos(qf[t,q], sf[t,c])

Design notes (final, ~67us/core vs 186us baseline):
  - All matmul operands are fp16 (1 cyc/row on the PE at any free-dim size,
    FWL weight loads, half the HBM bytes of f32). PSUM accumulation stays
    f32, as do all normalization scalars. ~0.14% rel err vs the 2% gate.
  - Host pre-packs the activation stream into per-column-group tensors that
    are contiguous per SBUF partition row (22 k-chunks x group-cols), so the
    whole 12.25MB/core stream moves in ~12 large DMAs (~9KB packets, near
    the 358GB/s HBM-per-core peak) and resides fully in SBUF.
  - Column groups stream in order [support | q0..q3]; each finished group's
    normalization/projection tail is emitted as a generator whose stages
    interleave between the next group's matmuls, so tails hide under the
    DMA-paced stream and never head-of-line-block an engine queue. The last
    groups' tails interleave op-by-op (q3 split in 256-col halves).
  - LayerNorm mean-subtraction and gains fold into weights host-side
    (column-centered weights); per-column inverse-std scales commute past
    the next projection (diag scaling commutes with left matmul); the
    Wo*bv constant enters via scalar_tensor_tensor. An all-ones [128,128]
    matmul both reduces sum(x^2) over hid and broadcasts it to all
    partitions; 1/sqrt comes from ACT Sqrt (pre-bias inputs, bias folded
    into the same op) + DVE reciprocal_approx_fast. Work is balanced
    DVE/ACT (squares + sqrts + PSUM bias-copies on ACT, muls/recips on DVE).
  - Scores: one matmul per 512-query chunk with 32 support columns
    stationary -> [32, 512] blocks scaled by 1/||f|| on output; the
    block-diagonal [q,c] extraction happens in the host-side unshard
    (pure indexing, no flops).
"""
import sys
sys.path.insert(0, "/opt/trn_rl_repo")
import numpy as np

HID = 128
T, Q, S = 256, 64, 4
DI, DTXT = 2048, 768
NCORES = 8
TPC = T // NCORES               # 32 tasks per core
FEAT = DI + DTXT                # 2816
KT = FEAT // 128                # 22 contraction chunks
QROWS = TPC * Q                 # 2048 query rows per core
SROWS = TPC * S                 # 128 support rows per core
GC = 512                        # query group columns
NQG = QROWS // GC               # 4 query groups
SCALE_INV = 1.0 / (np.sqrt(HID) + 1e-8)
EPS = 1e-5

_prog = None  # cached compiled Bass program


def _build():
    import concourse.bacc as bacc
    import concourse.tile as tile
    import concourse.mybir as mybir

    F32 = mybir.dt.float32
    F16 = mybir.dt.float16
    AFT = mybir.ActivationFunctionType
    AX = mybir.AxisListType
    ALU = mybir.AluOpType

    nc = bacc.Bacc()
    xs_d = nc.declare_dram_parameter("xs", [128, KT * SROWS], F16, isOutput=False)
    xq_d = [nc.declare_dram_parameter(f"xq{j}", [128, KT * GC], F16,
                                      isOutput=False) for j in range(NQG)]
    wc_d = nc.declare_dram_parameter("wc", [128, KT * HID], F16, isOutput=False)
    w4_d = nc.declare_dram_parameter("w4", [128, 4 * HID], F16, isOutput=False)
    bias_d = nc.declare_dram_parameter("biases", [HID, 8], F32, isOutput=False)
    mask_d = nc.declare_dram_parameter("mask", [SROWS, SROWS], F32, isOutput=False)
    id_d = nc.declare_dram_parameter("ident", [128, 128], F32, isOutput=False)
    v_d = nc.declare_dram_parameter("V", [32, QROWS], F32, isOutput=True)

    lp = nc.allow_low_precision(reason="fp16 matmul operands, f32 accumulation")
    lp.__enter__()

    with tile.TileContext(nc) as tc:
        with (
            tc.tile_pool(name="wts", bufs=1) as wts,
            tc.tile_pool(name="wk", bufs=4) as wk,
            tc.tile_pool(name="acc", bufs=2, space="PSUM") as accp,
            tc.tile_pool(name="accs", bufs=1, space="PSUM") as accsp,
            tc.tile_pool(name="tp", bufs=5, space="PSUM") as tpA,
        ):
            tpB = tpA
            # ---- persistent tiles ----
            wc_t = wts.tile([128, KT * HID], F16)
            xs_t = wts.tile([128, KT * SROWS], F16)
            xq_t = [wts.tile([128, KT * GC], F16, name=f"xq{j}")
                    for j in range(NQG)]
            w4_t = wts.tile([128, 4 * HID], F16)
            bias_t = wts.tile([HID, 8], F32)
            mask_t = wts.tile([SROWS, SROWS], F32)
            id_t = wts.tile([128, 128], F32)
            ones_t = wts.tile([128, 128], F16)
            sfn = wts.tile([128, SROWS], F16)
            v_sb = wts.tile([32, QROWS], F32)

            nc.vector.memset(ones_t, 1.0)
            eps_t = wts.tile([128, 4], F32)
            nc.vector.memset(eps_t[:, 0:1], EPS)
            nc.vector.memset(eps_t[:, 1:2], 1e-16)
            nc.vector.memset(eps_t[:, 2:3], 1e-18)
            nc.vector.memset(eps_t[:, 3:4], 0.0)

            # ---- DMA: split the stream across both HWDGE rings ----
            WH = 6 * HID
            SH = 6 * SROWS
            nc.sync.dma_start(out=wc_t[:, 0:WH], in_=wc_d[:, 0:WH])
            nc.sync.dma_start(out=xs_t[:, 0:SH], in_=xs_d[:, 0:SH])
            nc.sync.dma_start(out=wc_t[:, WH:KT * HID], in_=wc_d[:, WH:KT * HID])
            nc.sync.dma_start(out=xs_t[:, SH:KT * SROWS],
                              in_=xs_d[:, SH:KT * SROWS])
            nc.gpsimd.dma_start(out=w4_t, in_=w4_d[:])
            nc.gpsimd.dma_start(out=bias_t, in_=bias_d[:])
            nc.gpsimd.dma_start(out=mask_t, in_=mask_d[:])
            nc.gpsimd.dma_start(out=id_t, in_=id_d[:])
            HALF = KT * GC // 2
            for j in range(NQG):
                nc.sync.dma_start(out=xq_t[j][:, 0:HALF],
                                  in_=xq_d[j][:, 0:HALF])
                if j < NQG - 1:
                    nc.sync.dma_start(out=xq_t[j][:, HALF:KT * GC],
                                      in_=xq_d[j][:, HALF:KT * GC])
                else:
                    # split the last-arriving half so stream matmuls trail
                    # each piece's completion semaphore instead of one big one
                    for k0, k1 in ((11, 15), (15, 18), (18, 20), (20, 21),
                                   (21, 22)):
                        nc.sync.dma_start(
                            out=xq_t[j][:, k0 * GC:k1 * GC],
                            in_=xq_d[j][:, k0 * GC:k1 * GC])

            bc_t = bias_t[:, 0:1]
            bq_t = bias_t[:, 1:2]
            bk_t = bias_t[:, 2:3]
            bv_t = bias_t[:, 3:4]
            bo_t = bias_t[:, 4:5]
            wq_w = w4_t[:, 0 * HID:1 * HID]
            wk_w = w4_t[:, 1 * HID:2 * HID]
            wv_w = w4_t[:, 2 * HID:3 * HID]
            wo_w = w4_t[:, 3 * HID:4 * HID]

            def stream_group(acc_ps, x_t, cols, tail=None):
                for k in range(KT):
                    nc.tensor.matmul(acc_ps[:, :cols],
                                     wc_t[:, k * HID:(k + 1) * HID],
                                     x_t[:, k * cols:(k + 1) * cols],
                                     start=(k == 0), stop=(k == KT - 1))
                    if tail is not None:
                        next(tail, None)
                        next(tail, None)
                if tail is not None:
                    for _ in tail:
                        pass

            def rstd(src_f16, cn, scale, bias):
                """[128,cn] fp16 -> broadcast 1/sqrt(scale*colsum(src^2)+bias)."""
                sq = wk.tile([128, GC], F16, tag="sq")
                nc.vector.tensor_mul(out=sq[:, :cn], in0=src_f16[:, :cn],
                                     in1=src_f16[:, :cn])
                ss = tpA.tile([128, GC], F32, tag="tp")
                nc.tensor.matmul(ss[:, :cn], ones_t[:], sq[:, :cn],
                                 start=True, stop=True)
                sd = wk.tile([128, GC], F32, tag="sd")
                nc.scalar.activation(out=sd[:, :cn], in_=ss[:, :cn],
                                     func=AFT.Sqrt, bias=bias, scale=scale)
                r = wk.tile([128, GC], F32, tag="r")
                nc.vector.reciprocal_approx_fast(out=r[:, :cn], in_=sd[:, :cn])
                return r

            def query_tail(acc_ps, j, c0=0, cn=GC):
                """Generator: one chain stage per yield, for interleaving.
                Processes columns [c0, c0+cn) of query group j."""
                cb = wk.tile([128, GC], F16, tag="cb", name=f"cb{j}_{c0}")
                cb = cb[:, :cn]
                nc.scalar.activation(out=cb, in_=acc_ps[:, c0:c0 + cn],
                                     func=AFT.Identity, bias=bc_t, scale=1.0)
                yield
                sq = wk.tile([128, GC], F16, tag="sq", name=f"sq{j}_{c0}")
                sq = sq[:, :cn]
                nc.scalar.activation(out=sq, in_=acc_ps[:, c0:c0 + cn],
                                     func=AFT.Square, bias=bc_t, scale=1.0)
                yield
                ss = tpA.tile([128, GC], F32, tag="tp", name=f"ss1_{j}_{c0}")
                ss = ss[:, :cn]
                nc.tensor.matmul(ss, ones_t[:], sq, start=True, stop=True)
                u = tpB.tile([128, GC], F32, tag="tp", name=f"u{j}_{c0}")
                u = u[:, :cn]
                nc.tensor.matmul(u, wv_w, cb, start=True, stop=True)
                yield
                sd = wk.tile([128, GC], F32, tag="sd", name=f"sd1_{j}_{c0}")
                sd = sd[:, :cn]
                nc.scalar.activation(out=sd, in_=ss, func=AFT.Sqrt,
                                     bias=eps_t[:, 0:1], scale=1.0 / HID)
                yield
                r1 = wk.tile([128, GC], F32, tag="r", name=f"r1_{j}_{c0}")
                r1 = r1[:, :cn]
                nc.vector.reciprocal_approx_fast(out=r1, in_=sd)
                yield
                t1 = wk.tile([128, GC], F16, tag="t1", name=f"t1_{j}_{c0}")
                t1 = t1[:, :cn]
                nc.vector.tensor_mul(out=t1, in0=u, in1=r1)
                yield
                sq2 = wk.tile([128, GC], F16, tag="sq", name=f"sq2_{j}_{c0}")
                sq2 = sq2[:, :cn]
                nc.scalar.activation(out=sq2, in_=t1, func=AFT.Square,
                                     bias=bv_t, scale=1.0)
                yield
                ss2 = tpA.tile([128, GC], F32, tag="tp", name=f"ss2_{j}_{c0}")
                ss2 = ss2[:, :cn]
                nc.tensor.matmul(ss2, ones_t[:], sq2, start=True, stop=True)
                # Wo(t1+bv) = Wo t1 + wobv; the constant enters in the t2 stt
                s2 = tpB.tile([128, GC], F32, tag="tp", name=f"s2_{j}_{c0}")
                s2 = s2[:, :cn]
                nc.tensor.matmul(s2, wo_w, t1, start=True, stop=True)
                yield
                sd2 = wk.tile([128, GC], F32, tag="sd", name=f"sd2_{j}_{c0}")
                sd2 = sd2[:, :cn]
                nc.scalar.activation(out=sd2, in_=ss2, func=AFT.Sqrt,
                                     bias=eps_t[:, 0:1], scale=1.0 / HID)
                yield
                r2 = wk.tile([128, GC], F32, tag="r", name=f"r2_{j}_{c0}")
                r2 = r2[:, :cn]
                nc.vector.reciprocal_approx_fast(out=r2, in_=sd2)
                yield
                t2 = wk.tile([128, GC], F16, tag="t1", name=f"t2_{j}_{c0}")
                t2 = t2[:, :cn]
                nc.vector.scalar_tensor_tensor(
                    out=t2, in0=s2, scalar=bias_t[:, 7:8], in1=r2,
                    op0=ALU.add, op1=ALU.mult)
                yield
                f16 = wk.tile([128, GC], F16, tag="cb", name=f"f{j}_{c0}")
                f16 = f16[:, :cn]
                nc.vector.tensor_scalar_add(out=f16, in0=t2, scalar1=bo_t)
                yield
                sq3 = wk.tile([128, GC], F16, tag="sq", name=f"sq3_{j}_{c0}")
                sq3 = sq3[:, :cn]
                nc.scalar.activation(out=sq3, in_=t2, func=AFT.Square,
                                     bias=bo_t, scale=1.0)
                yield
                ss3 = tpA.tile([128, GC], F32, tag="tp", name=f"ss3_{j}_{c0}")
                ss3 = ss3[:, :cn]
                nc.tensor.matmul(ss3, ones_t[:], sq3, start=True, stop=True)
                yield
                sd3 = wk.tile([128, GC], F32, tag="sd", name=f"sd3_{j}_{c0}")
                sd3 = sd3[:, :cn]
                nc.scalar.activation(out=sd3, in_=ss3, func=AFT.Sqrt,
                                     bias=eps_t[:, 1:2], scale=1.0)
                yield
                r3 = wk.tile([128, GC], F32, tag="r", name=f"r3_{j}_{c0}")
                r3 = r3[:, :cn]
                nc.vector.reciprocal_approx_fast(out=r3, in_=sd3)
                yield
                # scores quad j: 32 support cols stationary vs 512 query cols;
                # the 1/||f|| column scale is applied on the [32,512] output
                vq = tpB.tile([128, GC], F32, tag="tp", name=f"vq{j}_{c0}")
                vq = vq[0:32, :cn]
                nc.tensor.matmul(vq, sfn[:, 32 * j:32 * j + 32],
                                 f16, start=True, stop=True)
                yield
                nc.vector.tensor_mul(
                    out=v_sb[:, GC * j + c0:GC * j + c0 + cn],
                    in0=vq, in1=r3[0:32, :])
                nc.sync.dma_start(out=v_d[:, GC * j + c0:GC * j + c0 + cn],
                                  in_=v_sb[:, GC * j + c0:GC * j + c0 + cn])

            def run_tails(*gens, step=1):
                gens = list(gens)
                while gens:
                    nxt = []
                    for g in gens:
                        alive = True
                        for _ in range(step):
                            if next(g, "END") == "END":
                                alive = False
                                break
                        if alive:
                            nxt.append(g)
                    gens = nxt

            # ---- support group: stream + full attention tail ----
            acc_s = accsp.tile([128, SROWS], F32, tag="accs")
            stream_group(acc_s, xs_t, SROWS)

            # ---- q0 stream (emitted before support tail: PE queue order) ----
            acc_q = [None] * NQG
            acc_q[0] = accp.tile([128, GC], F32, tag="accq", name="acc_q0")
            stream_group(acc_q[0], xq_t[0], GC)

            # ---- support tail (generator; interleaved into q1's stream) ----
            def support_tail():
                cn = SROWS
                cb_s = wk.tile([128, GC], F16, tag="cb")
                nc.vector.tensor_scalar_add(out=cb_s[:, :cn], in0=acc_s[:, :cn],
                                            scalar1=bc_t)
                yield
                sq_s = wk.tile([128, GC], F16, tag="sq")
                nc.vector.tensor_mul(out=sq_s[:, :cn], in0=cb_s[:, :cn],
                                     in1=cb_s[:, :cn])
                yield
                ss_s = tpA.tile([128, GC], F32, tag="tp")
                nc.tensor.matmul(ss_s[:, :cn], ones_t[:], sq_s[:, :cn],
                                 start=True, stop=True)
                yield
                sd_s = wk.tile([128, GC], F32, tag="sd")
                nc.scalar.activation(out=sd_s[:, :cn], in_=ss_s[:, :cn],
                                     func=AFT.Sqrt, bias=eps_t[:, 0:1],
                                     scale=1.0 / HID)
                yield
                r1_s = wk.tile([128, GC], F32, tag="r")
                nc.vector.reciprocal_approx_fast(out=r1_s[:, :cn],
                                                 in_=sd_s[:, :cn])
                yield
                h_s = wk.tile([128, SROWS], F16, tag="hs")
                nc.vector.tensor_mul(out=h_s, in0=cb_s[:, :cn],
                                     in1=r1_s[:, :cn])
                yield
                q_ps = tpB.tile([128, GC], F32, tag="tp")
                nc.tensor.matmul(q_ps[:, :cn], wq_w, h_s[:],
                                 start=True, stop=True)
                yield
                qT = wk.tile([128, SROWS], F16, tag="qT")
                nc.vector.tensor_scalar_add(out=qT, in0=q_ps[:, :cn],
                                            scalar1=bq_t)
                yield
                k_ps = tpB.tile([128, GC], F32, tag="tp")
                nc.tensor.matmul(k_ps[:, :cn], wk_w, h_s[:],
                                 start=True, stop=True)
                yield
                kT = wk.tile([128, SROWS], F16, tag="kT")
                nc.vector.tensor_scalar_add(out=kT, in0=k_ps[:, :cn],
                                            scalar1=bk_t)
                yield
                # v row-major: out[rows, hid] = h^T @ Wv^T  (lhsT = h)
                v_ps = tpA.tile([128, GC], F32, tag="tp")
                nc.tensor.matmul(v_ps[:, :HID], h_s[:], wv_w,
                                 start=True, stop=True)
                yield
                v16 = wk.tile([128, HID], F16, tag="v16")
                nc.vector.tensor_copy(out=v16, in_=v_ps[:, :HID])
                yield
                s_ps = tpB.tile([128, GC], F32, tag="tp")
                nc.tensor.matmul(s_ps[:, :cn], qT[:], kT[:],
                                 start=True, stop=True)
                yield
                sm = wk.tile([SROWS, SROWS], F32, tag="sm")
                nc.vector.tensor_add(out=sm, in0=s_ps[:, :cn], in1=mask_t)
                yield
                nmx = wk.tile([SROWS, 1], F32, tag="nmx")
                nc.vector.tensor_reduce(out=nmx, in_=sm, axis=AX.X,
                                        op=ALU.max, negate=True)
                yield
                asum = wk.tile([SROWS, 1], F32, tag="asum")
                a_f = wk.tile([SROWS, SROWS], F32, tag="af")
                nc.scalar.activation(out=a_f, in_=sm, func=AFT.Exp,
                                     bias=nmx, scale=1.0, accum_out=asum)
                yield
                rs = wk.tile([SROWS, 1], F32, tag="rs")
                nc.vector.reciprocal_approx_fast(out=rs, in_=asum)
                yield
                a2 = wk.tile([SROWS, SROWS], F32, tag="a2")
                nc.vector.tensor_scalar_mul(out=a2, in0=a_f, scalar1=rs)
                yield
                aT_ps = tpA.tile([128, GC], F32, tag="tp")
                nc.tensor.matmul(aT_ps[:, :cn], a2[:], id_t[:],
                                 is_transpose=True)
                yield
                aT16 = wk.tile([SROWS, SROWS], F16, tag="aT16")
                nc.vector.tensor_copy(out=aT16, in_=aT_ps[:, :cn])
                yield
                # ctx^T[hid, qrow] = v_rm^T @ a^T ; bv (centered) added after
                ctx_ps = tpB.tile([128, GC], F32, tag="tp")
                nc.tensor.matmul(ctx_ps[:, :cn], v16[:], aT16[:],
                                 start=True, stop=True)
                yield
                ctxb = wk.tile([128, SROWS], F16, tag="ctxb")
                nc.vector.tensor_scalar_add(out=ctxb, in0=ctx_ps[:, :cn],
                                            scalar1=bv_t)
                yield
                sq2_s = wk.tile([128, GC], F16, tag="sq")
                nc.vector.tensor_mul(out=sq2_s[:, :cn], in0=ctxb, in1=ctxb)
                yield
                ss2_s = tpA.tile([128, GC], F32, tag="tp")
                nc.tensor.matmul(ss2_s[:, :cn], ones_t[:], sq2_s[:, :cn],
                                 start=True, stop=True)
                yield
                sd2_s = wk.tile([128, GC], F32, tag="sd")
                nc.scalar.activation(out=sd2_s[:, :cn], in_=ss2_s[:, :cn],
                                     func=AFT.Sqrt, bias=eps_t[:, 0:1],
                                     scale=1.0 / HID)
                yield
                r2_s = wk.tile([128, GC], F32, tag="r")
                nc.vector.reciprocal_approx_fast(out=r2_s[:, :cn],
                                                 in_=sd2_s[:, :cn])
                yield
                s2_ps = tpB.tile([128, GC], F32, tag="tp")
                nc.tensor.matmul(s2_ps[:, :cn], wo_w, ctxb[:],
                                 start=True, stop=True)
                yield
                t2_s = wk.tile([128, SROWS], F32, tag="t2s")
                nc.vector.tensor_mul(out=t2_s, in0=s2_ps[:, :cn],
                                     in1=r2_s[:, :cn])
                yield
                f_s = wk.tile([128, SROWS], F32, tag="fs")
                nc.vector.tensor_scalar_add(out=f_s, in0=t2_s, scalar1=bo_t)
                yield
                sq3_s = wk.tile([128, GC], F16, tag="sq")
                nc.vector.tensor_mul(out=sq3_s[:, :cn], in0=f_s, in1=f_s)
                yield
                ss3_s = tpA.tile([128, GC], F32, tag="tp")
                nc.tensor.matmul(ss3_s[:, :cn], ones_t[:], sq3_s[:, :cn],
                                 start=True, stop=True)
                yield
                # 10/||f||: rsqrt(0.01*ss + 1e-18)
                sd3_s = wk.tile([128, GC], F32, tag="sd")
                nc.scalar.activation(out=sd3_s[:, :cn], in_=ss3_s[:, :cn],
                                     func=AFT.Sqrt, bias=eps_t[:, 2:3],
                                     scale=0.01)
                yield
                r3_s = wk.tile([128, GC], F32, tag="r")
                nc.vector.reciprocal_approx_fast(out=r3_s[:, :cn],
                                                 in_=sd3_s[:, :cn])
                yield
                nc.vector.tensor_mul(out=sfn, in0=f_s, in1=r3_s[:, :cn])

            # ---- remaining query groups; finished groups' tails interleave
            # into the next group's DMA-paced stream ----
            acc_q[1] = accp.tile([128, GC], F32, tag="accq", name="acc_q1")
            stream_group(acc_q[1], xq_t[1], GC, tail=support_tail())
            acc_q[2] = accp.tile([128, GC], F32, tag="accq", name="acc_q2")
            stream_group(acc_q[2], xq_t[2], GC, tail=query_tail(acc_q[0], 0))
            acc_q[3] = accp.tile([128, GC], F32, tag="accq", name="acc_q3")
            stream_group(acc_q[3], xq_t[3], GC, tail=query_tail(acc_q[1], 1))
            # interleave remaining tail chains op-by-op; q3 split in halves
            run_tails(query_tail(acc_q[2], 2),
                      query_tail(acc_q[3], 3, 0, 256),
                      query_tail(acc_q[3], 3, 256, 256))


    lp.__exit__(None, None, None)
    nc.compile()
    return nc


def _host_prep(inputs):
    f32, f16 = np.float32, np.float16
    Wi, Wt = np.asarray(inputs["Wi"], f32), np.asarray(inputs["Wt"], f32)
    bi, bt = np.asarray(inputs["bi"], f32), np.asarray(inputs["bt"], f32)
    g1, b1 = np.asarray(inputs["g1"], f32), np.asarray(inputs["b1"], f32)
    g2, b2 = np.asarray(inputs["g2"], f32), np.asarray(inputs["b2"], f32)
    Wq, bq = np.asarray(inputs["Wq"], f32), np.asarray(inputs["bq"], f32)
    Wk, bk = np.asarray(inputs["Wk"], f32), np.asarray(inputs["bk"], f32)
    Wv, bv = np.asarray(inputs["Wv"], f32), np.asarray(inputs["bv"], f32)
    Wo, bo = np.asarray(inputs["Wo"], f32), np.asarray(inputs["bo"], f32)

    Wc = np.concatenate([Wi, Wt], axis=1)          # [128, 2816]
    bc = bi + bt
    Wc_c = Wc - Wc.mean(axis=0, keepdims=True)     # fold LN1 mean
    bc_c = bc - bc.mean()

    Wq_f = (Wq * g1[None, :]) * SCALE_INV
    bq_f = (bq + Wq @ b1) * SCALE_INV
    Wk_f = Wk * g1[None, :]
    bk_f = bk + Wk @ b1
    Wv_f = Wv * g1[None, :]
    bv_f = bv + Wv @ b1
    Wv_c = Wv_f - Wv_f.mean(axis=0, keepdims=True)  # fold LN2 mean
    bv_c = bv_f - bv_f.mean()
    Wo_f = Wo * g2[None, :]
    bo_f = bo + Wo @ b2

    blk = np.arange(SROWS) // S
    mask = np.where(blk[:, None] == blk[None, :], 0.0, -1e30).astype(f32)

    # [hid_in, hid_out] orientation serves both lhsT and rhs roles
    w4 = np.concatenate([Wq_f.T, Wk_f.T, Wv_c.T, Wo_f.T], axis=1).astype(f16)
    biases = np.zeros((HID, 8), f32)
    biases[:, 0] = bc_c
    biases[:, 1] = bq_f
    biases[:, 2] = bk_f
    biases[:, 3] = bv_c
    biases[:, 4] = bo_f
    biases[:, 7] = Wo_f @ bv_c

    # wc packed [128, KT*HID]: partition p, col (k*HID+m) = Wc_c[m, 128k+p]
    wc_pack = np.ascontiguousarray(
        Wc_c.T.reshape(KT, 128, HID).transpose(1, 0, 2).reshape(128, KT * HID)
    ).astype(f16)

    common = {
        "wc": wc_pack, "w4": np.ascontiguousarray(w4),
        "biases": biases, "mask": mask,
        "ident": np.eye(128, dtype=f32),
    }

    si = np.asarray(inputs["support_images"], f32)
    st = np.asarray(inputs["support_texts"], f32)
    qi = np.asarray(inputs["query_images"], f32)
    qt = np.asarray(inputs["query_texts"], f32)

    def pack(xT, c0, cols):
        # xT [2816, rows] -> [128, KT*cols]: partition p, col (k*cols+c)
        #   = xT[128k+p, c0+c]; per-partition rows are contiguous per k-chunk
        return np.ascontiguousarray(
            xT.reshape(KT, 128, xT.shape[1])[:, :, c0:c0 + cols]
            .transpose(1, 0, 2).reshape(128, KT * cols))

    in_maps = []
    for m in range(NCORES):
        ts = slice(m * TPC, (m + 1) * TPC)
        Xs = np.concatenate([si[ts].reshape(SROWS, DI),
                             st[ts].reshape(SROWS, DTXT)], axis=1)
        Xq = np.concatenate([qi[ts].reshape(QROWS, DI),
                             qt[ts].reshape(QROWS, DTXT)], axis=1)
        xsT = Xs.T.astype(f16)                     # [2816, 128]
        xqT = Xq.T.astype(f16)                     # [2816, 2048]
        im = {"xs": pack(xsT, 0, SROWS), **common}
        for j in range(NQG):
            im[f"xq{j}"] = pack(xqT, j * GC, GC)
        in_maps.append(im)
    return in_maps


def _run(in_maps, trace=False, **kw):
    from concourse.bass_utils import run_bass_kernel_spmd
    global _prog
    if _prog is None:
        _prog = _build()
    return run_bass_kernel_spmd(_prog, in_maps, list(range(NCORES)),
                                trace=trace, **kw)


def _unshard(results):
    # V [32, 2048]: V[4i+c, 512j+64i+q] = logits[8j+i, q, c] for this core
    out = np.empty((T, Q, S), np.float32)
    ii = np.arange(8)
    for m in range(NCORES):
        V = results[m]["V"].reshape(8, 4, NQG, 8, Q)   # [i, c, j, i, q]
        blk = V[ii, :, :, ii, :]                       # [i, c, j, q]
        out[m * TPC:(m + 1) * TPC] = (
            blk.transpose(2, 0, 3, 1).reshape(TPC, Q, S))
    return out


def kernel(**inputs) -> np.ndarray:
    in_maps = _host_prep(inputs)
    res = _run(in_maps)
    return _unshard(res.results)
